# revision 7
# baseline (speedup 1.0000x reference)
"""Trainium2 Bass kernel for nn_MCAttention: 4 multi-scale cross-attention
blocks (q from context, k/v from detail, residual to detail).

Strategy:
- Pure data parallel: batch 16 -> 2 per core x 8 cores; weights replicated.
- Everything feature-major on chip ([E, tokens]); patch flatten/unflatten and
  all layout packing done host-side in numpy so every DMA is contiguous.
- Q/K head rows padded to 64-row slots, V/attention-output to 128-row slots
  (f32r matmuls may only write PSUM starting at partition 0), via host-side
  weight permutation; every per-head access is a legal aligned partition slice.
- Softmax without max subtraction (logits are tiny by construction); row sums
  come from a ones-column inside each V slot; normalization happens during
  PSUM eviction with the 1/sum row broadcast across partitions by a
  free-dim-step-0 SBUF->SBUF DMA.
- matmuls run in float32r (TF32-like, ~4x faster than fp32, ~1e-4/matmul
  error). The BIR verifier requires f32r matmul inputs to be produced by
  rounding-capable compute ops, so DMA-loaded x/weights pass through one DVE
  copy; scale-3 token counts are padded 49->50 (f32r moving width must be
  even); all matmul PSUM outputs start at partition 0.
"""

import numpy as np

P = 128
OSLOT = 128  # V / attention-output head slot (rows per head, psum dst at 0)

SCALES = [
    dict(C=96, H=56, W=56, s1=2, s2=2, N=784, E=384, d=48, slot=64, packb=False, tpad=784),
    dict(C=192, H=28, W=28, s1=1, s2=1, N=784, E=192, d=24, slot=64, packb=False, tpad=784),
    dict(C=384, H=14, W=14, s1=1, s2=1, N=196, E=384, d=48, slot=64, packb=True, tpad=196),
    dict(C=768, H=7, W=7, s1=1, s2=1, N=49, E=768, d=96, slot=128, packb=True, tpad=50),
]
HEADS = 8
BPC = 2
NCORES = 8
MMDT = "float32r"

_cache = {}


def _ceil_div(a, b):
    return (a + b - 1) // b


def _f_tiles(E):
    out = []
    r = E
    while r > 0:
        out.append(min(P, r))
        r -= P
    return out


def _col_splits(W, maxw=512):
    n = _ceil_div(W, maxw)
    base = _ceil_div(W, n)
    out = []
    s = 0
    while s < W:
        w = min(base, W - s)
        out.append((s, w))
        s += w
    return out


def _k_blocks(N):
    if N == 784:
        return [(i * 112, 112) for i in range(7)]
    if N == 196:
        return [(0, 98), (98, 98)]
    if N == 49:
        return [(0, 49)]
    raise ValueError(N)


def _build_program():
    import concourse.mybir as mybir
    import concourse.tile as tile
    from concourse import bacc
    from contextlib import ExitStack

    dt = mybir.dt
    nc = bacc.Bacc("TRN2", target_bir_lowering=False, debug=False,
                   num_devices=NCORES)

    dram = {}
    for i, cf in enumerate(SCALES):
        E = cf["E"]
        Wcols = (2 if cf["packb"] else 1) * cf["tpad"]
        Jf = _f_tiles(E)
        Mq = HEADS * cf["slot"] // P
        nb = 1 if cf["packb"] else BPC
        dram[f"xd{i}"] = nc.dram_tensor(f"xd{i}", [nb, P, len(Jf), Wcols],
                                        dt.float32, kind="ExternalInput")
        dram[f"xc{i}"] = nc.dram_tensor(f"xc{i}", [nb, P, len(Jf), Wcols],
                                        dt.float32, kind="ExternalInput")
        dram[f"wq{i}"] = nc.dram_tensor(f"wq{i}", [P, len(Jf), Mq * P],
                                        dt.float32, kind="ExternalInput")
        dram[f"wk{i}"] = nc.dram_tensor(f"wk{i}", [P, len(Jf), Mq * P],
                                        dt.float32, kind="ExternalInput")
        dram[f"wv{i}"] = nc.dram_tensor(f"wv{i}", [P, len(Jf), E],
                                        dt.float32, kind="ExternalInput")
        dram[f"wo{i}"] = nc.dram_tensor(f"wo{i}", [P, HEADS, E],
                                        dt.float32, kind="ExternalInput")
        dram[f"out{i}"] = nc.dram_tensor(f"out{i}", [nb, P, len(Jf), Wcols],
                                         dt.float32, kind="ExternalOutput")

    with tile.TileContext(nc) as tc:
        for i, cf in enumerate(SCALES):
            _emit_scale(tc, nc, mybir, dram, i, cf)

    nc.compile()
    return nc


def _emit_scale(tc, nc, mybir, dram, i, cf):
    import concourse.bass as bass
    from contextlib import ExitStack

    dt = mybir.dt
    DT = getattr(dt, MMDT)
    AFT = mybir.ActivationFunctionType
    E, N, d, slot = cf["E"], cf["N"], cf["d"], cf["slot"]
    tpad = cf["tpad"]
    packb = cf["packb"]
    Jf = _f_tiles(E)
    nJ = len(Jf)
    Mq = HEADS * slot // P          # q/k padded row tiles
    spt = P // slot                 # q/k head slots per 128-row tile
    Wcols = (2 if packb else 1) * tpad
    nb = 1 if packb else BPC
    csp = _col_splits(Wcols)        # projection moving splits (even widths)
    qsp = _col_splits(tpad)         # per-batch attention q splits
    kbs = _k_blocks(N)

    def mm(out_ap, lhsT, rhs, start, stop):
        nc.tensor.matmul(out_ap, lhsT, rhs, start=start, stop=stop)

    with ExitStack() as ctx:
        stpool = ctx.enter_context(tc.tile_pool(name=f"st{i}", bufs=1))
        wpool = ctx.enter_context(tc.tile_pool(name=f"w{i}", bufs=1))
        xpool = ctx.enter_context(tc.tile_pool(name=f"x{i}", bufs=min(nb, 2)))
        xcpool = ctx.enter_context(tc.tile_pool(name=f"xc{i}", bufs=1))
        qkpool = ctx.enter_context(tc.tile_pool(name=f"qk{i}", bufs=1))
        vpool = ctx.enter_context(
            tc.tile_pool(name=f"v{i}", bufs=len(kbs) * (2 if packb else 1) + 1))
        ptpool = ctx.enter_context(
            tc.tile_pool(name=f"pt{i}", bufs=len(kbs) + 1))
        otpool = ctx.enter_context(tc.tile_pool(name=f"ot{i}", bufs=1))
        recpool = ctx.enter_context(tc.tile_pool(name=f"rec{i}", bufs=2))
        ypool = ctx.enter_context(tc.tile_pool(name=f"y{i}", bufs=3))
        ps_proj = ctx.enter_context(
            tc.tile_pool(name=f"psp{i}", bufs=2, space="PSUM"))
        ps_st = ctx.enter_context(
            tc.tile_pool(name=f"pst{i}", bufs=2, space="PSUM"))
        ps_pv = ctx.enter_context(
            tc.tile_pool(name=f"ppv{i}", bufs=2, space="PSUM"))

        def staged_load(pool, shape, tag, src):
            """DMA fp32 -> stage, DVE round-copy -> f32r tile."""
            st = stpool.tile(shape, dt.float32, tag="stage")
            nc.sync.dma_start(out=st[:], in_=src)
            t = pool.tile(shape, DT, tag=tag)
            nc.vector.tensor_copy(t[:], st[:])
            return t

        wq = staged_load(wpool, [P, nJ, Mq * P], "wq", dram[f"wq{i}"][:])
        wk = staged_load(wpool, [P, nJ, Mq * P], "wk", dram[f"wk{i}"][:])
        wv = staged_load(wpool, [P, nJ, E], "wv", dram[f"wv{i}"][:])
        wo = staged_load(wpool, [P, HEADS, E], "wo", dram[f"wo{i}"][:])

        for b in range(nb):
            xd = staged_load(xpool, [P, nJ, Wcols], "xd", dram[f"xd{i}"][b])
            xc = staged_load(xcpool, [P, nJ, Wcols], "xc", dram[f"xc{i}"][b])

            # --- Q, K projections (64-row head slots, feature-major) ---
            qt = qkpool.tile([P, Mq, Wcols], DT, tag="qt")
            kt = qkpool.tile([P, Mq, Wcols], DT, tag="kt")
            for (w_t, x_t, o_t) in ((wq, xc, qt), (wk, xd, kt)):
                for m in range(Mq):
                    for (c0, cw) in csp:
                        psm = ps_proj.tile([P, 512], dt.float32, tag="psp")
                        for j in range(nJ):
                            r = Jf[j]
                            mm(psm[:, 0:cw],
                               w_t[0:r, j, m * P:(m + 1) * P],
                               x_t[0:r, j, c0:c0 + cw],
                               j == 0, j == nJ - 1)
                        nc.scalar.activation(
                            o_t[:, m, c0:c0 + cw], psm[:, 0:cw], AFT.Copy)

            # --- V projection into per-k-block aug tiles [kb, H, OSLOT] ---
            # per-head slot: cols 0..d-1 = V rows, col OSLOT-1 = ones,
            # cols d..OSLOT-2 never read
            ones16 = ypool.tile([P, 16], dt.float32, tag="ones16")
            nc.any.memset(ones16[:], 1.0)
            vaug = {}
            for bb in range(2 if packb else 1):
                t0 = bb * tpad
                for (k0, kw) in kbs:
                    va = vpool.tile([P, HEADS, OSLOT], DT, tag="vaug")
                    nc.vector.tensor_copy(
                        va[0:kw, :, OSLOT - 2:OSLOT],
                        ones16[0:kw, :].rearrange("p (h x) -> p h x", h=HEADS))
                    for (e0, ew) in _col_splits(E):
                        psv = ps_proj.tile([P, 512], dt.float32, tag="psp")
                        for j in range(nJ):
                            r = Jf[j]
                            mm(psv[0:kw, 0:ew],
                               xd[0:r, j, t0 + k0:t0 + k0 + kw],
                               wv[0:r, j, e0:e0 + ew],
                               j == 0, j == nJ - 1)
                        h0, nh = e0 // d, ew // d
                        pv3 = psv[0:kw, 0:ew].rearrange(
                            "p (h x) -> p h x", h=nh)
                        nc.vector.tensor_copy(va[0:kw, h0:h0 + nh, 0:d], pv3)
                    vaug[(bb, k0)] = va

            # --- attention (per batch-slot, per head, per q-chunk) ---
            ot = otpool.tile([P, HEADS, Wcols], DT, tag="ot")
            for bb in range(2 if packb else 1):
                t0 = bb * tpad
                for h in range(HEADS):
                    jh = h // spt
                    base = slot * (h % spt)
                    for (q0, qw) in qsp:
                        pts = []
                        for (k0, kw) in kbs:
                            pss = ps_st.tile([112, 392], dt.float32,
                                             tag="pst")
                            mm(pss[0:kw, 0:qw],
                               kt[base:base + d, jh, t0 + k0:t0 + k0 + kw],
                               qt[base:base + d, jh, t0 + q0:t0 + q0 + qw],
                               True, True)
                            pt = ptpool.tile([112, 392], DT, tag="pt")
                            nc.scalar.activation(pt[0:kw, 0:qw],
                                                 pss[0:kw, 0:qw], AFT.Exp)
                            pts.append((k0, kw, pt))
                        ppv = ps_pv.tile([P, 392], dt.float32, tag="ppv")
                        for ki, (k0, kw, pt) in enumerate(pts):
                            va = vaug[(bb, k0)]
                            mm(ppv[:, 0:qw],
                               va[0:kw, h, :],
                               pt[0:kw, 0:qw],
                               ki == 0, ki == len(pts) - 1)
                        rec = recpool.tile([P, 392], dt.float32, tag="rec")
                        nc.vector.reciprocal(rec[96:128, 0:qw],
                                             ppv[96:128, 0:qw])
                        bc = recpool.tile([P, 392], dt.float32, tag="bc")
                        src = rec[OSLOT - 2:OSLOT - 1, 0:qw]
                        src_b = bass.AP(src.tensor, src.offset,
                                        [src.ap[0], [0, d], src.ap[1]])
                        nc.sync.dma_start(out=bc[0:d, 0:qw], in_=src_b)
                        nc.vector.tensor_mul(
                            ot[0:d, h, t0 + q0:t0 + q0 + qw],
                            ppv[0:d, 0:qw], bc[0:d, 0:qw])

            # --- O projection + residual ---
            for m in range(nJ):
                rm = Jf[m]
                for (c0, cw) in csp:
                    psy = ps_proj.tile([P, 512], dt.float32, tag="psp")
                    for j in range(HEADS):
                        mm(psy[0:rm, 0:cw],
                           wo[0:d, j, m * P:m * P + rm],
                           ot[0:d, j, c0:c0 + cw],
                           j == 0, j == HEADS - 1)
                    yt = ypool.tile([P, 512], dt.float32, tag="yt")
                    nc.vector.tensor_add(
                        yt[0:rm, 0:cw], psy[0:rm, 0:cw],
                        xd[0:rm, m, c0:c0 + cw].bitcast(dt.float32))
                    nc.sync.dma_start(
                        out=dram[f"out{i}"][b, 0:rm, m, c0:c0 + cw],
                        in_=yt[0:rm, 0:cw])


# ---------------- host-side data prep ----------------

def _flatten_np(x, s1, s2):
    # b c (h s1) (w s2) -> b (s1 s2 c) (h w)   [feature-major]
    b, c, H, W = x.shape
    h, w = H // s1, W // s2
    x = x.reshape(b, c, h, s1, w, s2).transpose(0, 3, 5, 1, 2, 4)
    return np.ascontiguousarray(x.reshape(b, s1 * s2 * c, h * w))


def _unflatten_np(y, c, h, w, s1, s2):
    b = y.shape[0]
    y = y.reshape(b, s1, s2, c, h, w).transpose(0, 3, 4, 1, 5, 2)
    return np.ascontiguousarray(y.reshape(b, c, h * s1, w * s2))


def _pack_ftiles(a2d):
    """[F, M] -> [P, nJ, M] with f = j*128 + p (pad rows zero)."""
    F, M = a2d.shape
    nJ = _ceil_div(F, P)
    out = np.zeros((P, nJ, M), np.float32)
    for j in range(nJ):
        r = min(P, F - j * P)
        out[0:r, j, :] = a2d[j * P:j * P + r, :]
    return np.ascontiguousarray(out)


def _prep_wmaps(inputs):
    wmaps = {}
    for i, cf in enumerate(SCALES):
        E, d, slot = cf["E"], cf["d"], cf["slot"]
        Mq = HEADS * slot // P * P
        scale = float(d) ** -0.5
        wq = np.asarray(inputs[f"wq{i}"], np.float32)
        wk = np.asarray(inputs[f"wk{i}"], np.float32)
        wv = np.asarray(inputs[f"wv{i}"], np.float32)
        wo = np.asarray(inputs[f"wo{i}"], np.float32)
        wqT = np.zeros((E, Mq), np.float32)
        wkT = np.zeros((E, Mq), np.float32)
        for h in range(HEADS):
            wqT[:, slot * h:slot * h + d] = wq[d * h:d * h + d, :].T * scale
            wkT[:, slot * h:slot * h + d] = wk[d * h:d * h + d, :].T
        wmaps[f"wq{i}"] = _pack_ftiles(wqT)
        wmaps[f"wk{i}"] = _pack_ftiles(wkT)
        wmaps[f"wv{i}"] = _pack_ftiles(np.ascontiguousarray(wv.T))
        # wo: contraction over OSLOT-row head slots: [P, HEADS, E] with
        # rows 0..d-1 of head-slot j = wo[:, d*j:d*j+d].T
        woT = np.zeros((P, HEADS, E), np.float32)
        for h in range(HEADS):
            woT[0:d, h, :] = wo[:, d * h:d * h + d].T
        wmaps[f"wo{i}"] = np.ascontiguousarray(woT)
    return wmaps


def _prep_xmaps(inputs):
    names = ["x00", "x10", "x20", "x30"]
    xs = {}
    for i, cf in enumerate(SCALES):
        xdf = _flatten_np(np.asarray(inputs[names[i] + "_detail"], np.float32),
                          cf["s1"], cf["s2"])
        xcf = _flatten_np(np.asarray(inputs[names[i] + "_context"], np.float32),
                          cf["s1"], cf["s2"])
        xs[i] = (xdf, xcf)
    return xs


def _core_inputs(xs, wmaps, c):
    m = dict(wmaps)
    for i, cf in enumerate(SCALES):
        E, N, tpad = cf["E"], cf["N"], cf["tpad"]
        xdf, xcf = xs[i]
        sl = slice(2 * c, 2 * c + 2)
        if cf["packb"]:
            xd2 = np.zeros((E, 2, tpad), np.float32)
            xc2 = np.zeros((E, 2, tpad), np.float32)
            xd2[:, :, 0:N] = xdf[sl].transpose(1, 0, 2)
            xc2[:, :, 0:N] = xcf[sl].transpose(1, 0, 2)
            m[f"xd{i}"] = _pack_ftiles(xd2.reshape(E, 2 * tpad))[None]
            m[f"xc{i}"] = _pack_ftiles(xc2.reshape(E, 2 * tpad))[None]
        else:
            m[f"xd{i}"] = np.stack(
                [_pack_ftiles(xdf[2 * c + b]) for b in range(2)])
            m[f"xc{i}"] = np.stack(
                [_pack_ftiles(xcf[2 * c + b]) for b in range(2)])
    return m


def kernel(**inputs):
    if "nc" not in _cache:
        _cache["nc"] = _build_program()
    nc = _cache["nc"]

    for i in range(4):
        for p in "qkvo":
            b = np.asarray(inputs[f"b{p}{i}"], np.float32)
            assert (b == 0).all(), "kernel assumes zero attention biases"

    wmaps = _prep_wmaps(inputs)
    xs = _prep_xmaps(inputs)
    in_maps = [_core_inputs(xs, wmaps, c) for c in range(NCORES)]

    from concourse.bass_utils import run_bass_kernel_spmd
    res = run_bass_kernel_spmd(nc, in_maps, list(range(NCORES)))

    outs = []
    for i, cf in enumerate(SCALES):
        C, H, W, s1, s2 = cf["C"], cf["H"], cf["W"], cf["s1"], cf["s2"]
        E, N, tpad = cf["E"], cf["N"], cf["tpad"]
        nJ = len(_f_tiles(E))
        full = np.empty((16, E, N), np.float32)
        for c in range(NCORES):
            o = res.results[c][f"out{i}"]
            if cf["packb"]:
                y = o[0].transpose(1, 0, 2).reshape(nJ * P, 2, tpad)
                for b in range(2):
                    full[2 * c + b] = y[0:E, b, 0:N]
            else:
                for b in range(2):
                    full[2 * c + b] = o[b].transpose(1, 0, 2).reshape(
                        nJ * P, tpad)[0:E, 0:N]
        outs.append(_unflatten_np(full, C, H // s1, W // s2, s1, s2))
    return tuple(outs)
